# revision 11
# baseline (speedup 1.0000x reference)
"""L21 norm kernel for Trainium2 (Bass/Tile), 8-core SPMD.

Computes sum_j sqrt(sum_i S[i,j]^2) for S of shape [8192, 16384] fp32.

Sharding: S is split along columns into 8 shards of [8192, 2048] (one per
NeuronCore). Each core computes the sum of its columns' L2 norms as a
scalar; the host sums the 8 partial scalars.

Per-core dataflow (memory-bound; 64 MiB HBM read per core):
  - Bulk: 15 tiles of [128 partitions, 4 rows, 2048 cols] fp32 (4 MiB
    HWDGE DMAs; each partition's slice is 32 KiB contiguous in DRAM),
    then one [128, 2, 2048] tile and two [128, 1, 2048] sub-tiles so the
    post-last-byte chain stays short.
  - ACT engine: square with bf16 output (also the dtype cast for PE).
  - Partition-axis reduction is split so neither engine paces the DMA
    stream: per bulk tile, row-slices q=0,1 go to PE (ones[128,1]^T @ sq
    matmuls accumulating into PSUM [1,2048] fp32) and q=2,3 are
    accumulated on DVE into a bf16 [128,2048] accumulator (2x mode),
    folded into PSUM every 5 tiles (short bf16 chains for accuracy; PE
    has mid-stream slack). The final tiles go straight to PE.
  - Epilogue: per-512-block ACT sqrt with accum_out producing the block
    sums in the same instruction, one tiny DVE reduce, DMA out.
"""

import numpy as np

# Full problem shape (hardcoded per the harness contract).
R = 8192          # rows
C_FULL = 16384    # columns
N_CORES = 8
C = C_FULL // N_CORES  # 2048 columns per core
P = 128           # SBUF partitions
NBLK = 512        # matmul moving free dim (one PSUM bank of fp32)

T4 = 15           # bulk tiles: [P, 4, C], rows 0..7680
ROWS4 = T4 * P * 4
# DVE-accumulator fold points (after the adds at tile t) and the tiles
# that restart the accumulator with a copy.
FOLD_TILES = (4, 9, T4 - 1)
RESET_TILES = (5, 10)

_cached = None


def _build():
    """Build + schedule the per-core Bass program. Returns the Bacc object."""
    import concourse.bacc as bacc
    import concourse.tile as tile
    from concourse import mybir

    nc = bacc.Bacc(
        "TRN2",
        target_bir_lowering=False,
        debug=False,
        enable_asserts=False,
        num_devices=N_CORES,
    )

    s_dram = nc.dram_tensor("S", [R, C], mybir.dt.float32, kind="ExternalInput")
    out_dram = nc.dram_tensor("out", [1, 1], mybir.dt.float32, kind="ExternalOutput")

    s_ap = s_dram.ap()
    out_ap = out_dram.ap()

    # Bulk view [T4, P, 4, C]: partition p holds 4 consecutive rows ->
    # 32 KiB contiguous DRAM per (t, p) descriptor.
    v4 = s_ap[:ROWS4, :].rearrange("(t p q) c -> t p q c", p=P, q=4)
    # Tail: four [P, C] sub-tiles (1 MiB each) so the ACT engine keeps
    # pace with DMA delivery at the end of the stream.
    v1 = s_ap[ROWS4:, :].rearrange("(s p) c -> s p c", p=P)

    with tile.TileContext(nc) as tc:
        with (
            tc.tile_pool(name="io", bufs=4) as io_pool,
            tc.tile_pool(name="sqp", bufs=3) as sq_pool,
            tc.tile_pool(name="const", bufs=1) as const_pool,
            tc.tile_pool(name="ps", bufs=1, space="PSUM") as ps_pool,
            tc.tile_pool(name="fin", bufs=1) as fin_pool,
        ):
            # First input DMA before any const setup so streaming starts as
            # early as possible.
            x0 = io_pool.tile([P, 4, C], mybir.dt.float32, tag="x")
            # Issued from the ACT engine's HWDGE ring: its preamble clears
            # earlier than Sync's, so streaming starts sooner.
            nc.scalar.dma_start(out=x0, in_=v4[0])

            ones = const_pool.tile([P, 1], mybir.dt.bfloat16)
            nc.vector.memset(ones, 1.0)

            # DVE-side accumulator for q=2,3 row-slices.
            acc = const_pool.tile([P, C], mybir.dt.bfloat16)

            # Per-column sum of squares (4 PSUM banks).
            colsq = ps_pool.tile([1, C], mybir.dt.float32)

            # Dummy sqrt: pulls the sqrt ACT-table load out of the tail.
            warm = const_pool.tile([1, 1], mybir.dt.float32)
            nc.scalar.sqrt(out=warm, in_=ones[0:1, :])

            def pe_reduce(src, first, last):
                for b in range(C // NBLK):
                    nc.tensor.matmul(
                        colsq[:, b * NBLK : (b + 1) * NBLK],
                        ones,
                        src[:, b * NBLK : (b + 1) * NBLK],
                        start=first,
                        stop=(last and b == C // NBLK - 1),
                    )

            for t in range(T4):
                if t == 0:
                    x_tile = x0
                else:
                    x_tile = io_pool.tile([P, 4, C], mybir.dt.float32, tag="x")
                    nc.sync.dma_start(out=x_tile, in_=v4[t])

                sq = sq_pool.tile([P, 4, C], mybir.dt.bfloat16, tag="sq")
                nc.scalar.square(out=sq, in_=x_tile)

                pe_reduce(sq[:, 0, :], first=(t == 0), last=False)
                pe_reduce(sq[:, 1, :], first=False, last=False)

                if t == 0 or t in RESET_TILES:
                    nc.vector.tensor_copy(acc, sq[:, 2, :])
                else:
                    nc.vector.tensor_add(acc, acc, sq[:, 2, :])
                nc.vector.tensor_add(acc, acc, sq[:, 3, :])

                if t in FOLD_TILES:
                    pe_reduce(acc, first=False, last=False)

            # Tail sub-tiles (all-PE; the accumulator is already folded).
            for s in range(4):
                xs = io_pool.tile([P, 1, C], mybir.dt.float32, tag="x")
                nc.sync.dma_start(out=xs[:, 0, :], in_=v1[s])
                sqs = sq_pool.tile([P, 1, C], mybir.dt.bfloat16, tag="sq")
                nc.scalar.square(out=sqs, in_=xs)
                pe_reduce(sqs[:, 0, :], first=False, last=(s == 3))

            # One sqrt over all columns; accum_out produces the sum of the
            # norms in the same ACT instruction -> no DVE in the tail.
            norms = fin_pool.tile([1, C], mybir.dt.float32)
            total = fin_pool.tile([1, 1], mybir.dt.float32)
            nc.scalar.activation(
                norms,
                colsq,
                mybir.ActivationFunctionType.Sqrt,
                accum_out=total,
            )

            nc.sync.dma_start(out=out_ap, in_=total)

    nc.compile()
    return nc


def _get_nc():
    global _cached
    if _cached is None:
        _cached = _build()
    return _cached


def _run(S: np.ndarray, trace: bool = False):
    from concourse import bass_utils

    assert S.shape == (R, C_FULL), S.shape
    S = np.ascontiguousarray(np.asarray(S, dtype=np.float32))

    nc = _get_nc()
    in_maps = [
        {"S": np.ascontiguousarray(S[:, i * C : (i + 1) * C])} for i in range(N_CORES)
    ]
    try:
        res = bass_utils.run_bass_kernel_spmd(
            nc, in_maps, core_ids=list(range(N_CORES)), trace=trace
        )
    except Exception:
        # One retry: transient NRT/device hiccups (e.g. a wedged core from a
        # previous process) are recoverable on re-execution.
        res = bass_utils.run_bass_kernel_spmd(
            nc, in_maps, core_ids=list(range(N_CORES)), trace=trace
        )
    partials = np.array(
        [res.results[i]["out"][0, 0] for i in range(N_CORES)], dtype=np.float64
    )
    out = np.float32(partials.sum())
    return out, res


def kernel(S: np.ndarray) -> np.ndarray:
    out, _ = _run(S, trace=False)
    return np.asarray(out, dtype=np.float32)


def run_traced(S: np.ndarray):
    """For test.py: returns (output, BassKernelResults) with NTFF trace."""
    return _run(S, trace=True)


# revision 12
# speedup vs baseline: 1.0571x; 1.0571x over previous
"""L21 norm kernel for Trainium2 (Bass/Tile), 8-core SPMD.

Computes sum_j sqrt(sum_i S[i,j]^2) for S of shape [8192, 16384] fp32.

Sharding: S is split along columns into 8 shards of [8192, 2048] (one per
NeuronCore). Each core computes the sum of its columns' L2 norms as a
scalar; the host sums the 8 partial scalars.

Per-core dataflow (memory-bound; 64 MiB HBM read per core):
  - Bulk: 15 tiles of [128 partitions, 4 rows, 2048 cols] fp32 (4 MiB
    HWDGE DMAs; each partition's slice is 32 KiB contiguous in DRAM),
    then four [128, 1, 2048] sub-tiles so the ACT engine keeps pace with
    delivery and the post-last-byte chain stays short.
  - ACT engine: square with bf16 output (also the dtype cast for PE).
  - Partition-axis reduction is split so neither engine paces the DMA
    stream: per bulk tile, row-slices q=0,1 go to PE (ones[128,1]^T @ sq
    matmuls accumulating into PSUM [1,2048] fp32) and q=2,3 are
    accumulated on DVE into a bf16 [128,2048] accumulator (2x mode),
    folded into PSUM every 5 tiles (short bf16 chains for accuracy; PE
    has mid-stream slack). The tail sub-tiles go straight to PE.
  - Epilogue: one ACT sqrt over [1, 2048] whose accum_out emits the sum
    of the norms in the same instruction, then DMA out.
"""

import numpy as np

# Full problem shape (hardcoded per the harness contract).
R = 8192          # rows
C_FULL = 16384    # columns
N_CORES = 8
C = C_FULL // N_CORES  # 2048 columns per core
P = 128           # SBUF partitions
NBLK = 512        # matmul moving free dim (one PSUM bank of fp32)

T4 = 15           # bulk tiles: [P, 4, C], rows 0..7680
ROWS4 = T4 * P * 4
# DVE-accumulator fold points (after the adds at tile t) and the tiles
# that restart the accumulator with a copy.
FOLD_TILES = (4, 9, T4 - 1)
RESET_TILES = (5, 10)

_cached = None


def _build():
    """Build + schedule the per-core Bass program. Returns the Bacc object."""
    import concourse.bacc as bacc
    import concourse.tile as tile
    from concourse import mybir

    nc = bacc.Bacc(
        "TRN2",
        target_bir_lowering=False,
        debug=False,
        enable_asserts=False,
        num_devices=N_CORES,
    )

    s_dram = nc.dram_tensor("S", [R, C], mybir.dt.float32, kind="ExternalInput")
    out_dram = nc.dram_tensor("out", [1, 1], mybir.dt.float32, kind="ExternalOutput")

    s_ap = s_dram.ap()
    out_ap = out_dram.ap()

    # Bulk view [T4, P, 4, C]: partition p holds 4 consecutive rows ->
    # 32 KiB contiguous DRAM per (t, p) descriptor.
    v4 = s_ap[:ROWS4, :].rearrange("(t p q) c -> t p q c", p=P, q=4)
    # Tail: four [P, C] sub-tiles (1 MiB each) so the ACT engine keeps
    # pace with DMA delivery at the end of the stream.
    v1 = s_ap[ROWS4:, :].rearrange("(s p) c -> s p c", p=P)

    with tile.TileContext(nc) as tc:
        with (
            tc.tile_pool(name="io", bufs=4) as io_pool,
            tc.tile_pool(name="sqp", bufs=3) as sq_pool,
            tc.tile_pool(name="const", bufs=1) as const_pool,
            tc.tile_pool(name="ps", bufs=1, space="PSUM") as ps_pool,
            tc.tile_pool(name="fin", bufs=1) as fin_pool,
        ):
            # First input DMA before any const setup so streaming starts as
            # early as possible.
            x0 = io_pool.tile([P, 4, C], mybir.dt.float32, tag="x")
            # Issued from the ACT engine's HWDGE ring: its preamble clears
            # earlier than Sync's, so streaming starts sooner.
            nc.scalar.dma_start(out=x0, in_=v4[0])

            ones = const_pool.tile([P, 1], mybir.dt.bfloat16)
            nc.vector.memset(ones, 1.0)

            # DVE-side accumulator for q=2,3 row-slices.
            acc = const_pool.tile([P, C], mybir.dt.bfloat16)

            # Per-column sum of squares (4 PSUM banks).
            colsq = ps_pool.tile([1, C], mybir.dt.float32)

            # Dummy sqrt: pulls the sqrt ACT-table load out of the tail.
            warm = const_pool.tile([1, 1], mybir.dt.float32)
            nc.scalar.sqrt(out=warm, in_=ones[0:1, :])

            def pe_reduce(src, first, last):
                for b in range(C // NBLK):
                    nc.tensor.matmul(
                        colsq[:, b * NBLK : (b + 1) * NBLK],
                        ones,
                        src[:, b * NBLK : (b + 1) * NBLK],
                        start=first,
                        stop=(last and b == C // NBLK - 1),
                    )

            for t in range(T4):
                if t == 0:
                    x_tile = x0
                else:
                    x_tile = io_pool.tile([P, 4, C], mybir.dt.float32, tag="x")
                    nc.sync.dma_start(out=x_tile, in_=v4[t])

                sq = sq_pool.tile([P, 4, C], mybir.dt.bfloat16, tag="sq")
                nc.scalar.square(out=sq, in_=x_tile)

                pe_reduce(sq[:, 0, :], first=(t == 0), last=False)
                pe_reduce(sq[:, 1, :], first=False, last=False)

                if t == 0 or t in RESET_TILES:
                    nc.vector.tensor_copy(acc, sq[:, 2, :])
                else:
                    nc.vector.tensor_add(acc, acc, sq[:, 2, :])
                nc.vector.tensor_add(acc, acc, sq[:, 3, :])

                if t in FOLD_TILES:
                    pe_reduce(acc, first=False, last=False)

            # Tail sub-tiles (all-PE; the accumulator is already folded).
            for s in range(4):
                xs = io_pool.tile([P, 1, C], mybir.dt.float32, tag="x")
                nc.sync.dma_start(out=xs[:, 0, :], in_=v1[s])
                sqs = sq_pool.tile([P, 1, C], mybir.dt.bfloat16, tag="sq")
                nc.scalar.square(out=sqs, in_=xs)
                pe_reduce(sqs[:, 0, :], first=False, last=(s == 3))

            # One sqrt over all columns; accum_out produces the sum of the
            # norms in the same ACT instruction -> no DVE in the tail.
            norms = fin_pool.tile([1, C], mybir.dt.float32)
            total = fin_pool.tile([1, 1], mybir.dt.float32)
            nc.scalar.activation(
                norms,
                colsq,
                mybir.ActivationFunctionType.Sqrt,
                accum_out=total,
            )

            nc.sync.dma_start(out=out_ap, in_=total)

    nc.compile()
    return nc


def _get_nc():
    global _cached
    if _cached is None:
        _cached = _build()
    return _cached


def _run(S: np.ndarray, trace: bool = False):
    from concourse import bass_utils

    assert S.shape == (R, C_FULL), S.shape
    S = np.ascontiguousarray(np.asarray(S, dtype=np.float32))

    nc = _get_nc()
    in_maps = [
        {"S": np.ascontiguousarray(S[:, i * C : (i + 1) * C])} for i in range(N_CORES)
    ]
    try:
        res = bass_utils.run_bass_kernel_spmd(
            nc, in_maps, core_ids=list(range(N_CORES)), trace=trace
        )
    except Exception:
        # One retry: transient NRT/device hiccups (e.g. a wedged core from a
        # previous process) are recoverable on re-execution.
        res = bass_utils.run_bass_kernel_spmd(
            nc, in_maps, core_ids=list(range(N_CORES)), trace=trace
        )
    partials = np.array(
        [res.results[i]["out"][0, 0] for i in range(N_CORES)], dtype=np.float64
    )
    out = np.float32(partials.sum())
    return out, res


def kernel(S: np.ndarray) -> np.ndarray:
    out, _ = _run(S, trace=False)
    return np.asarray(out, dtype=np.float32)


def run_traced(S: np.ndarray):
    """For test.py: returns (output, BassKernelResults) with NTFF trace."""
    return _run(S, trace=True)


# revision 14
# speedup vs baseline: 1.2476x; 1.1803x over previous
"""L21 norm kernel for Trainium2 (Bass/Tile), 8-core SPMD.

Computes sum_j sqrt(sum_i S[i,j]^2) for S of shape [8192, 16384] fp32.

Sharding: S is split along columns into 8 shards of [8192, 2048] (one per
NeuronCore). Each core computes the sum of its columns' L2 norms as a
scalar; the host sums the 8 partial scalars.

Per-core dataflow (memory-bound; 64 MiB HBM read per core):
  - Bulk: 15 tiles of [128 partitions, 4 rows, 2048 cols] fp32 (4 MiB
    HWDGE DMAs; each partition's slice is 32 KiB contiguous in DRAM),
    then four [128, 1, 2048] sub-tiles so the ACT engine keeps pace with
    delivery and the post-last-byte chain stays short.
  - ACT engine: square with bf16 output (also the dtype cast for PE).
  - Partition-axis reduction is split so neither engine paces the DMA
    stream: per bulk tile, row-slices q=0,1 go to PE (ones[128,1]^T @ sq
    matmuls accumulating into PSUM [1,2048] fp32) and q=2,3 are
    accumulated on DVE into a bf16 [128,2048] accumulator (2x mode),
    folded into PSUM every 5 tiles (short bf16 chains for accuracy; PE
    has mid-stream slack). The tail sub-tiles go straight to PE.
  - Epilogue: one ACT sqrt over [1, 2048] whose accum_out emits the sum
    of the norms in the same instruction, then DMA out.
"""

import numpy as np

# Full problem shape (hardcoded per the harness contract).
R = 8192          # rows
C_FULL = 16384    # columns
N_CORES = 8
C = C_FULL // N_CORES  # 2048 columns per core
P = 128           # SBUF partitions
NBLK = 512        # matmul moving free dim (one PSUM bank of fp32)

T4 = 15           # bulk tiles: [P, 4, C], rows 0..7680
ROWS4 = T4 * P * 4
# DVE-accumulator fold points (after the adds at tile t) and the tiles
# that restart the accumulator with a copy.
FOLD_TILES = (4, 9, T4 - 1)
RESET_TILES = (5, 10)

_cached = None


def _build():
    """Build + schedule the per-core Bass program. Returns the Bacc object."""
    import concourse.bacc as bacc
    import concourse.tile as tile
    from concourse import mybir

    nc = bacc.Bacc(
        "TRN2",
        target_bir_lowering=False,
        debug=False,
        enable_asserts=False,
        num_devices=N_CORES,
    )

    s_dram = nc.dram_tensor("S", [R, C], mybir.dt.float32, kind="ExternalInput")
    out_dram = nc.dram_tensor("out", [1, 1], mybir.dt.float32, kind="ExternalOutput")

    s_ap = s_dram.ap()
    out_ap = out_dram.ap()

    # Bulk view [T4, P, 4, C]: partition p holds 4 consecutive rows ->
    # 32 KiB contiguous DRAM per (t, p) descriptor.
    v4 = s_ap[:ROWS4, :].rearrange("(t p q) c -> t p q c", p=P, q=4)
    # Tail: four [P, C] sub-tiles (1 MiB each) so the ACT engine keeps
    # pace with DMA delivery at the end of the stream.
    v1 = s_ap[ROWS4:, :].rearrange("(s p) c -> s p c", p=P)

    with tile.TileContext(nc) as tc:
        with (
            tc.tile_pool(name="io", bufs=4) as io_pool,
            tc.tile_pool(name="sqp", bufs=3) as sq_pool,
            tc.tile_pool(name="const", bufs=1) as const_pool,
            tc.tile_pool(name="ps", bufs=1, space="PSUM") as ps_pool,
            tc.tile_pool(name="fin", bufs=1) as fin_pool,
        ):
            # First input DMA before any const setup so streaming starts as
            # early as possible.
            x0 = io_pool.tile([P, 4, C], mybir.dt.float32, tag="x")
            # Issued from the ACT engine's HWDGE ring: its preamble clears
            # earlier than Sync's, so streaming starts sooner.
            nc.scalar.dma_start(out=x0, in_=v4[0])

            ones = const_pool.tile([P, 1], mybir.dt.bfloat16)
            nc.vector.memset(ones, 1.0)

            # DVE-side accumulator for q=2,3 row-slices.
            acc = const_pool.tile([P, C], mybir.dt.bfloat16)

            # Per-column sum of squares (4 PSUM banks).
            colsq = ps_pool.tile([1, C], mybir.dt.float32)

            # Dummy sqrt: pulls the sqrt ACT-table load out of the tail.
            warm = const_pool.tile([1, 1], mybir.dt.float32)
            nc.scalar.sqrt(out=warm, in_=ones[0:1, :])

            def pe_reduce(src, first, last):
                for b in range(C // NBLK):
                    nc.tensor.matmul(
                        colsq[:, b * NBLK : (b + 1) * NBLK],
                        ones,
                        src[:, b * NBLK : (b + 1) * NBLK],
                        start=first,
                        stop=(last and b == C // NBLK - 1),
                    )

            for t in range(T4):
                if t == 0:
                    x_tile = x0
                else:
                    x_tile = io_pool.tile([P, 4, C], mybir.dt.float32, tag="x")
                    nc.sync.dma_start(out=x_tile, in_=v4[t])

                sq = sq_pool.tile([P, 4, C], mybir.dt.bfloat16, tag="sq")
                nc.scalar.square(out=sq, in_=x_tile)

                pe_reduce(sq[:, 0, :], first=(t == 0), last=False)
                pe_reduce(sq[:, 1, :], first=False, last=False)

                if t == 0 or t in RESET_TILES:
                    nc.vector.tensor_copy(acc, sq[:, 2, :])
                else:
                    nc.vector.tensor_add(acc, acc, sq[:, 2, :])
                nc.vector.tensor_add(acc, acc, sq[:, 3, :])

                if t in FOLD_TILES:
                    pe_reduce(acc, first=False, last=False)

            # Tail sub-tiles (all-PE; the accumulator is already folded).
            for s in range(4):
                xs = io_pool.tile([P, 1, C], mybir.dt.float32, tag="x")
                nc.sync.dma_start(out=xs[:, 0, :], in_=v1[s])
                sqs = sq_pool.tile([P, 1, C], mybir.dt.bfloat16, tag="sq")
                nc.scalar.square(out=sqs, in_=xs)
                pe_reduce(sqs[:, 0, :], first=False, last=(s == 3))

            # One sqrt over all columns; accum_out produces the sum of the
            # norms in the same ACT instruction -> no DVE in the tail.
            norms = fin_pool.tile([1, C], mybir.dt.float32)
            total = fin_pool.tile([1, 1], mybir.dt.float32)
            nc.scalar.activation(
                norms,
                colsq,
                mybir.ActivationFunctionType.Sqrt,
                accum_out=total,
            )

            nc.sync.dma_start(out=out_ap, in_=total)

    nc.compile()
    return nc


def _get_nc():
    global _cached
    if _cached is None:
        _cached = _build()
    return _cached


def _run(S: np.ndarray, trace: bool = False):
    from concourse import bass_utils

    assert S.shape == (R, C_FULL), S.shape
    S = np.ascontiguousarray(np.asarray(S, dtype=np.float32))

    nc = _get_nc()
    in_maps = [
        {"S": np.ascontiguousarray(S[:, i * C : (i + 1) * C])} for i in range(N_CORES)
    ]
    try:
        res = bass_utils.run_bass_kernel_spmd(
            nc, in_maps, core_ids=list(range(N_CORES)), trace=trace
        )
    except Exception:
        # One retry: transient NRT/device hiccups (e.g. a wedged core from a
        # previous process) are recoverable on re-execution.
        res = bass_utils.run_bass_kernel_spmd(
            nc, in_maps, core_ids=list(range(N_CORES)), trace=trace
        )
    partials = np.array(
        [res.results[i]["out"][0, 0] for i in range(N_CORES)], dtype=np.float64
    )
    out = np.float32(partials.sum())
    return out, res


def kernel(S: np.ndarray) -> np.ndarray:
    out, _ = _run(S, trace=False)
    return np.asarray(out, dtype=np.float32)


def run_traced(S: np.ndarray):
    """For test.py: returns (output, BassKernelResults) with NTFF trace."""
    return _run(S, trace=True)


# revision 16
# speedup vs baseline: 1.2495x; 1.0015x over previous
"""L21 norm kernel for Trainium2 (Bass/Tile), 8-core SPMD.

Computes sum_j sqrt(sum_i S[i,j]^2) for S of shape [8192, 16384] fp32.

Sharding: S is split along columns into 8 shards of [8192, 2048] (one per
NeuronCore). Each core computes the sum of its columns' L2 norms as a
scalar; the host sums the 8 partial scalars.

Per-core dataflow (memory-bound; 64 MiB HBM read per core):
  - Bulk: 15 tiles of [128 partitions, 4 rows, 2048 cols] fp32 (4 MiB
    HWDGE DMAs; each partition's slice is 32 KiB contiguous in DRAM),
    then four [128, 1, 2048] sub-tiles so the ACT engine keeps pace with
    delivery and the post-last-byte chain stays short.
  - ACT engine: square with bf16 output (also the dtype cast for PE).
  - Partition-axis reduction is split so neither engine paces the DMA
    stream: per bulk tile, row-slices q=0,1 go to PE (ones[128,1]^T @ sq
    matmuls accumulating into PSUM [1,2048] fp32) and q=2,3 are
    accumulated on DVE into a bf16 [128,2048] accumulator (2x mode),
    folded into PSUM every 5 tiles (short bf16 chains for accuracy; PE
    has mid-stream slack). The tail sub-tiles go straight to PE.
  - Epilogue: one ACT sqrt over [1, 2048] whose accum_out emits the sum
    of the norms in the same instruction, then DMA out.
"""

import numpy as np

# Full problem shape (hardcoded per the harness contract).
R = 8192          # rows
C_FULL = 16384    # columns
N_CORES = 8
C = C_FULL // N_CORES  # 2048 columns per core
P = 128           # SBUF partitions
NBLK = 512        # matmul moving free dim (one PSUM bank of fp32)

T4 = 15           # bulk tiles: [P, 4, C], rows 0..7680
ROWS4 = T4 * P * 4
# DVE-accumulator fold points (after the adds at tile t) and the tiles
# that restart the accumulator with a copy.
FOLD_TILES = (4, 9, T4 - 1)
RESET_TILES = (5, 10)

_cached = None


def _build():
    """Build + schedule the per-core Bass program. Returns the Bacc object."""
    import concourse.bacc as bacc
    import concourse.tile as tile
    from concourse import mybir

    nc = bacc.Bacc(
        "TRN2",
        target_bir_lowering=False,
        debug=False,
        enable_asserts=False,
        num_devices=N_CORES,
    )

    s_dram = nc.dram_tensor("S", [R, C], mybir.dt.float32, kind="ExternalInput")
    out_dram = nc.dram_tensor("out", [1, 1], mybir.dt.float32, kind="ExternalOutput")

    s_ap = s_dram.ap()
    out_ap = out_dram.ap()

    # Bulk view [T4, P, 4, C]: partition p holds 4 consecutive rows ->
    # 32 KiB contiguous DRAM per (t, p) descriptor.
    v4 = s_ap[:ROWS4, :].rearrange("(t p q) c -> t p q c", p=P, q=4)
    # Tail: four [P, C] sub-tiles (1 MiB each) so the ACT engine keeps
    # pace with DMA delivery at the end of the stream.
    v1 = s_ap[ROWS4:, :].rearrange("(s p) c -> s p c", p=P)

    with tile.TileContext(nc) as tc:
        with (
            tc.tile_pool(name="io", bufs=4) as io_pool,
            tc.tile_pool(name="sqp", bufs=3) as sq_pool,
            tc.tile_pool(name="const", bufs=1) as const_pool,
            tc.tile_pool(name="ps", bufs=1, space="PSUM") as ps_pool,
            tc.tile_pool(name="fin", bufs=1) as fin_pool,
        ):
            # First input DMA before any const setup so streaming starts as
            # early as possible.
            x0 = io_pool.tile([P, 4, C], mybir.dt.float32, tag="x")
            # Issued from the ACT engine's HWDGE ring: its preamble clears
            # earlier than Sync's, so streaming starts sooner.
            nc.scalar.dma_start(out=x0, in_=v4[0])

            ones = const_pool.tile([P, 1], mybir.dt.bfloat16)
            nc.vector.memset(ones, 1.0)

            # DVE-side accumulator for q=2,3 row-slices.
            acc = const_pool.tile([P, C], mybir.dt.bfloat16)

            # Per-column sum of squares (4 PSUM banks).
            colsq = ps_pool.tile([1, C], mybir.dt.float32)

            # Dummy sqrt: pulls the sqrt ACT-table load out of the tail.
            warm = const_pool.tile([1, 1], mybir.dt.float32)
            nc.scalar.sqrt(out=warm, in_=ones[0:1, :])

            def pe_reduce(src, first, last):
                for b in range(C // NBLK):
                    nc.tensor.matmul(
                        colsq[:, b * NBLK : (b + 1) * NBLK],
                        ones,
                        src[:, b * NBLK : (b + 1) * NBLK],
                        start=first,
                        stop=(last and b == C // NBLK - 1),
                    )

            for t in range(T4):
                if t == 0:
                    x_tile = x0
                else:
                    x_tile = io_pool.tile([P, 4, C], mybir.dt.float32, tag="x")
                    nc.sync.dma_start(out=x_tile, in_=v4[t])

                sq = sq_pool.tile([P, 4, C], mybir.dt.bfloat16, tag="sq")
                nc.scalar.square(out=sq, in_=x_tile)

                pe_reduce(sq[:, 0, :], first=(t == 0), last=False)
                pe_reduce(sq[:, 1, :], first=False, last=False)

                if t == 0 or t in RESET_TILES:
                    nc.vector.tensor_copy(acc, sq[:, 2, :])
                else:
                    nc.vector.tensor_add(acc, acc, sq[:, 2, :])
                nc.vector.tensor_add(acc, acc, sq[:, 3, :])

                if t in FOLD_TILES:
                    pe_reduce(acc, first=False, last=False)

            # Tail sub-tiles (all-PE; the accumulator is already folded).
            for s in range(4):
                xs = io_pool.tile([P, 1, C], mybir.dt.float32, tag="x")
                nc.sync.dma_start(out=xs[:, 0, :], in_=v1[s])
                sqs = sq_pool.tile([P, 1, C], mybir.dt.bfloat16, tag="sq")
                nc.scalar.square(out=sqs, in_=xs)
                pe_reduce(sqs[:, 0, :], first=False, last=(s == 3))

            # One sqrt over all columns; accum_out produces the sum of the
            # norms in the same ACT instruction -> no DVE in the tail.
            norms = fin_pool.tile([1, C], mybir.dt.float32)
            total = fin_pool.tile([1, 1], mybir.dt.float32)
            nc.scalar.activation(
                norms,
                colsq,
                mybir.ActivationFunctionType.Sqrt,
                accum_out=total,
            )

            nc.sync.dma_start(out=out_ap, in_=total)

    nc.compile()
    return nc


def _get_nc():
    global _cached
    if _cached is None:
        _cached = _build()
    return _cached


def _run(S: np.ndarray, trace: bool = False):
    from concourse import bass_utils

    assert S.shape == (R, C_FULL), S.shape
    S = np.ascontiguousarray(np.asarray(S, dtype=np.float32))

    nc = _get_nc()
    in_maps = [
        {"S": np.ascontiguousarray(S[:, i * C : (i + 1) * C])} for i in range(N_CORES)
    ]
    try:
        res = bass_utils.run_bass_kernel_spmd(
            nc, in_maps, core_ids=list(range(N_CORES)), trace=trace
        )
    except Exception:
        # One retry: transient NRT/device hiccups (e.g. a wedged core from a
        # previous process) are recoverable on re-execution.
        res = bass_utils.run_bass_kernel_spmd(
            nc, in_maps, core_ids=list(range(N_CORES)), trace=trace
        )
    partials = np.array(
        [res.results[i]["out"][0, 0] for i in range(N_CORES)], dtype=np.float64
    )
    out = np.float32(partials.sum())
    return out, res


def kernel(S: np.ndarray) -> np.ndarray:
    out, _ = _run(S, trace=False)
    return np.asarray(out, dtype=np.float32)


def run_traced(S: np.ndarray):
    """For test.py: returns (output, BassKernelResults) with NTFF trace."""
    return _run(S, trace=True)
